# revision 26
# baseline (speedup 1.0000x reference)
"""DGCNN Bass/Tile kernel for Trainium2 — 8-core data-parallel (1 point cloud per core).

Per edge-conv block (exact algebra):
  edge feat [ctr, nbr] @ W = ctr @ Wc + nbr @ Wn
  out[n] = max_k relu(bn(A[n] + B[idx[n,k]])) = relu((A[n] + max_k B[idx[n,k]]) * s + t)
  (s = g*rsqrt(v+eps) > 0, t = b - m*s; relu/max/affine commute since s > 0)

k-NN scores (monotone-equivalent to the reference's pd, per row):
  score[n, m] = 2<x_n, x_m> - |x_m|^2    (row-constant -|x_n|^2 dropped)
computed on PE (fp32) into PSUM, evicted by ACT.

Top-16 selection reads each 2048-wide row exactly TWICE on DVE:
  pass 1: 8 chunk-local max8 ops (256-wide) -> candidate values cv [128, 64]
  pass 2: 8 chunk-local max_index ops       -> chunk-local indices ci
Then everything runs on the 64-wide candidate arrays: gi = ci + 256*chunk + 1
(fp32); max8(cv) -> top-8 values; match_replace marks their cv positions;
(cv != replaced) * gi -> max8 extracts the global indices themselves; repeat
once on the replaced array for ranks 9-16. Exact unless one 256-chunk holds
>8 of a row's top-16 (verified < 3e-3 final rel err on the fixed dataset).

The pd scores rotate through FOUR quarter-width PSUM buffers: ACT evicts
quarter q while the PE fills later quarters / the next tile, and each 512-col
eviction unblocks its two DVE scan chunks immediately (a single pd buffer
serialized PE vs ACT every tile). The gi-add and mask-multiply steps of the
index extraction run on GPSIMD (only add/mult tensor_tensor are walrus-legal
on Pool). The final concat matmul runs entirely in fp16 (x4 holders and fp16
copies of x1/x2/x3 feed only this 1x1 conv, so 4.9e-4 rounding is harmless
and fp16 matmuls stream at 1 cyc/row vs 4 for fp32); block 4's k=16 slot-max
is an fp16 pairwise max tree in the DVE 2x mode.

Gather of B = x @ Wn rows from DRAM via gpsimd dma_gather (int16 idx wrapped
into 16 partitions, replicated to the 8 Q7 cores). Wrap batches of 6/5/5
tiles are software-pipelined one batch ahead: batch k's gathers run while
batch k+1's tiles are scanned, and the 5-tile TAIL batches keep the last
gathers' latency hidden under earlier consumes at each block boundary
(measured better than 8/4/2/1/1, 6/4/3/3, 4x4x4x4, and host-precomputed
block-1 prep). The next batch's first two pd tiles are pre-emitted ahead of
the pending consumes so their PSUM evictions don't queue behind the
consumes' BN work in ACT program order.
Block 4's B table is fp16 (its features feed no further neighbor selection -
only the final 1x1 conv); blocks 1-3 keep fp32 B + tensor_reduce, since
feature precision feeds the next kNN graph.

NOTE for future optimization: float32r looks like a free 4x on wide fp32
matmuls in the cost model, and fp32r matmul arithmetic is bit-identical to
fp32 given the same input bytes - BUT every producer instruction must be
tagged float32r, and tagged writes (ACT *and* DMA) physically round the
stored values to ~2.4e-4 (tf32-like). That noise flips ~12% of rows' marginal
kNN picks (exact fp32 small-K scores flip ~none) and compounds to 7.7e-2
final error. Selection-feeding data can never pass through f32r storage.

Epilogue: the slot-max transpose PSUM-accumulates with A^T = Wc^T @ x^T
recomputed per tile (no DVE add, no persistent A^T buffer); fused BN+relu on
ACT writes the next block's x^T holder directly. The next block's -|x|^2 row
and 2x^T operands are produced per 512-column slice as soon as the epilogues
covering that slice land, and block 4's epilogue is chased per-tile by the
final concat matmul, so block boundaries stay tight.
"""
import numpy as np
from contextlib import ExitStack

import concourse.bass as bass
import concourse.mybir as mybir
import concourse.tile as tile
from concourse import bacc
from concourse import bass_utils
from concourse.masks import make_identity

N = 2048
K = 16
EPS = 1e-3
NT = N // 128  # 16 row-tiles
# large tail batches: the last gathers' latency hides under the previous
# tiles' consumes instead of being fully exposed at each block boundary;
# the 6-buffer gather pool self-throttles the in-flight depth
BATCHES = [[0, 1, 2, 3, 4, 5], [6, 7, 8, 9, 10, 11, 12, 13, 14, 15]]
BLOCKS = [(3, 64), (64, 64), (64, 128), (128, 256)]  # (C_in, D_out)

F32 = mybir.dt.float32
F16 = mybir.dt.float16
U16 = mybir.dt.uint16
I16 = mybir.dt.int16
AF = mybir.ActivationFunctionType
ALU = mybir.AluOpType


def build(nc: bass.Bass):
    # ---- DRAM I/O ----
    xT_d = nc.dram_tensor("xT", [3, N], F32, kind="ExternalInput")
    wc_d, wn_d, s_d, t_d = [], [], [], []
    for i, (C, D) in enumerate(BLOCKS):
        wc_d.append(nc.dram_tensor(f"Wc{i+1}", [C, D], F32, kind="ExternalInput"))
        wn_d.append(nc.dram_tensor(f"Wn{i+1}", [C, D], F32, kind="ExternalInput"))
        s_d.append(nc.dram_tensor(f"s{i+1}", [D, 1], F32, kind="ExternalInput"))
        t_d.append(nc.dram_tensor(f"t{i+1}", [D, 1], F32, kind="ExternalInput"))
    w5_d = nc.dram_tensor("W5c", [128, 5 * 512], F16, kind="ExternalInput")
    t5_d = nc.dram_tensor("t5", [1, 512], F16, kind="ExternalInput")
    ones16_d = nc.dram_tensor("ones16", [1, 128], F16, kind="ExternalInput")
    out_d = nc.dram_tensor("out", [N, 512], F32, kind="ExternalOutput")
    b_d = [nc.dram_tensor(f"bdram{i+1}", [N, D], F16 if i == 3 else F32,
                          kind="Internal")
           for i, (C, D) in enumerate(BLOCKS)]

    with tile.TileContext(nc) as tc, ExitStack() as ctx:
        sb = ctx.enter_context(tc.tile_pool(name="sb", bufs=2))
        sb1 = ctx.enter_context(tc.tile_pool(name="sb1", bufs=1))
        ps = ctx.enter_context(tc.tile_pool(name="ps", bufs=2, space="PSUM"))
        psd = ctx.enter_context(tc.tile_pool(name="psd", bufs=4, space="PSUM"))
        sb3 = ctx.enter_context(tc.tile_pool(name="sb3", bufs=6))

        ident = sb1.tile([128, 128], F32, tag="ident")
        make_identity(nc, ident[:])
        ident16 = sb1.tile([128, 128], F16, tag="ident16")
        nc.scalar.activation(ident16[:], ident[:], AF.Copy)
        ones_row = sb1.tile([1, N], F32, tag="ones_row")
        nc.gpsimd.memset(ones_row[:], 1.0)
        ones_col = sb1.tile([128, 1], F32, tag="ones_col")
        nc.gpsimd.memset(ones_col[:], 1.0)
        # candidate -> global index offset: off[c*8+s] = 256*c + 1
        off_c = sb1.tile([128, 8, 8], F32, tag="off_c")
        nc.gpsimd.iota(off_c[:], [[256, 8], [0, 8]], base=1,
                       channel_multiplier=0,
                       allow_small_or_imprecise_dtypes=True)

        # persistent x^T holders for block4 output (final concat matmul only
        # -> fp16; the 4.9e-4 rounding feeds nothing downstream but the 1x1)
        x4aT = sb1.tile([128, N], F16, tag="x4aT")
        x4bT = sb1.tile([128, N], F16, tag="x4bT")

        # per-block Cc holders: rows 0:C = x^T, row C = -sq (block4: sq4)
        cc1 = sb1.tile([4, N], F32, tag="cc1")
        cc2 = sb1.tile([65, N], F32, tag="cc2")
        cc3 = sb1.tile([65, N], F32, tag="cc3")
        cc4 = sb1.tile([128, N], F32, tag="cc4")
        sq4 = sb1.tile([1, N], F32, tag="sq4")
        # fp16 copies of x1/x2/x3 for the final concat matmul (values only
        # feed the 1x1 conv; 4.9e-4 rounding is harmless, fp16 matmul is
        # 1 cyc/row vs 4 for fp32)
        cch2 = sb1.tile([64, N], F16, tag="cch2")
        cch3 = sb1.tile([64, N], F16, tag="cch3")
        cch4 = sb1.tile([128, N], F16, tag="cch4")

        nc.sync.dma_start(cc1[0:3, :], xT_d.ap())
        w5 = sb1.tile([128, 5, 512], F16, tag="w5")
        nc.sync.dma_start(w5[:], w5_d.ap().rearrange("p (a d) -> p a d", a=5))
        t5 = sb1.tile([1, 512], F16, tag="t5")
        nc.sync.dma_start(t5[:], t5_d.ap())
        ones16 = sb1.tile([1, 128], F16, tag="ones16")
        nc.sync.dma_start(ones16[:], ones16_d.ap())

        xT_of = {1: cc1, 2: cc2, 3: cc3, 4: cc4}
        cch_of = {2: cch2, 3: cch3, 4: cch4}
        kchunks = [(cch2, 64), (cch3, 64), (cch4, 128), (x4aT, 128), (x4bT, 128)]

        def make_prep(blk):
            """Per-512-col-slice prep of block blk's pd operands: the -|x|^2
            row (DMA'd into the aug row) and rr = 2x^T."""
            C, D = BLOCKS[blk - 1]
            cct = xT_of[blk]
            xT = cct[0:C, :]
            rr = sb1.tile([C + 1, N] if blk < 4 else [128, N], F32,
                          tag=f"rr{blk % 2}")
            if blk < 4:
                nc.sync.dma_start(rr[C:C + 1, :], ones_row[:])

            def emit_slice(j):
                sl = slice(j * 512, (j + 1) * 512)
                xsq = sb.tile([C, 512], F32, tag="xsq")
                nc.scalar.activation(xsq[:], xT[:, sl], AF.Square)
                sqp = ps.tile([1, 512], F32, tag="pscratch")
                nc.tensor.matmul(sqp[:], ones_col[0:C, :], xsq[:],
                                 start=True, stop=True)
                sqstage = sb.tile([1, 512], F32, tag="sqstage")
                nc.scalar.activation(sqstage[:], sqp[:], AF.Copy, scale=-1.0)
                sq_dst = sq4[0:1, sl] if blk == 4 else cct[C:C + 1, sl]
                nc.sync.dma_start(sq_dst, sqstage[:])
                nc.scalar.mul(rr[0:C, sl], xT[:, sl], 2.0)
                if blk > 1:
                    nc.scalar.activation(cch_of[blk][0:C, sl], xT[:, sl], AF.Copy)

            return rr, emit_slice

        prepped = {1: make_prep(1)}
        for j in range(4):
            prepped[1][1](j)

        for i, (C, D) in enumerate(BLOCKS):
            blk = i + 1
            cct = xT_of[blk]
            xT = cct[0:C, :]
            bd_ap = b_d[i].ap()
            bdt = F16 if blk == 4 else F32
            rr = prepped[blk][0]

            if blk < 4:
                rr_chunks = [rr[0:C + 1, :]]
                cc_chunks = [cct[0:C + 1, :]]
            else:
                rr_chunks = [rr[:], ones_row[:]]
                cc_chunks = [cct[0:128, :], sq4[:]]
            nk = len(rr_chunks)

            def emit_pd(t):
                # four quarter-width PSUM buffers: ACT evicts quarter q while
                # the PE already fills the next quarters (and the next tile) —
                # a single buffer would serialize PE vs ACT every tile, and
                # each 512-col eviction unblocks its two scan chunks early
                pdsb = sb.tile([128, N], F32, tag="pdsb")
                for q in range(4):
                    pd = psd.tile([128, 512], F32, tag="pd")
                    for kc, (rc, cc) in enumerate(zip(rr_chunks, cc_chunks)):
                        nc.tensor.matmul(
                            pd[:],
                            rc[:, t * 128:(t + 1) * 128],
                            cc[:, q * 512:(q + 1) * 512],
                            start=(kc == 0), stop=(kc == nk - 1))
                    nc.scalar.activation(pdsb[:, q * 512:(q + 1) * 512],
                                         pd[:], AF.Copy)
                return pdsb

            # hoist pd of the first two tiles ahead of the weight/B prep so
            # the DVE scan starts while PE/ACT still run the block prep
            pre = {t: emit_pd(t) for t in (0, 1)}

            # ---- prep: weights / bn params ----
            wc = sb.tile([C, D], F32, tag="wc")
            wn = sb.tile([C, D], F32, tag="wn")
            nc.sync.dma_start(wc[:], wc_d[i].ap())
            nc.sync.dma_start(wn[:], wn_d[i].ap())
            nch = (D + 127) // 128
            s_sb = sb.tile([128, nch], F32, tag="s_sb")
            t_sb = sb.tile([128, nch], F32, tag="t_sb")
            for c in range(nch):
                dw_ = min(128, D - c * 128)
                nc.sync.dma_start(s_sb[0:dw_, c:c + 1], s_d[i].ap()[c * 128:c * 128 + dw_, :])
                nc.sync.dma_start(t_sb[0:dw_, c:c + 1], t_d[i].ap()[c * 128:c * 128 + dw_, :])

            # ---- prep: B = x @ Wn row-major -> DRAM (block4: fp16) ----
            for t in range(NT):
                bp = ps.tile([128, D], F32, tag="pscratch")
                nc.tensor.matmul(bp[:], xT[:, t * 128:(t + 1) * 128], wn[:],
                                 start=True, stop=True)
                b_sb = sb.tile([128, D], bdt, tag="b_sb")
                nc.scalar.activation(b_sb[:], bp[:], AF.Copy)
                nc.sync.dma_start(bd_ap[t * 128:(t + 1) * 128, :], b_sb[:])

            # ---- main loop ----
            if blk == 1:
                dsts = [(cc2, 0)]
            elif blk == 2:
                dsts = [(cc3, 0)]
            elif blk == 3:
                dsts = [(cc4, 0)]
            else:
                dsts = [(x4aT, 0), (x4bT, 0)]

            def scan(t, itile):
                pdsb = pre.pop(t, None)
                if pdsb is None:
                    pdsb = emit_pd(t)
                # pass 1+2: per-chunk top-8 values + chunk-local indices
                cv = sb.tile([128, 64], F32, tag="cv")
                ci = sb.tile([128, 64], U16, tag="ci")
                for c in range(8):
                    nc.vector.max(out=cv[:, c * 8:(c + 1) * 8],
                                  in_=pdsb[:, c * 256:(c + 1) * 256])
                    nc.vector.max_index(out=ci[:, c * 8:(c + 1) * 8],
                                        in_max=cv[:, c * 8:(c + 1) * 8],
                                        in_values=pdsb[:, c * 256:(c + 1) * 256])
                # gi = global index + 1, as exact fp32
                gi = sb.tile([128, 64], F32, tag="gi")
                nc.scalar.activation(gi[:], ci[:], AF.Copy)
                nc.gpsimd.tensor_tensor(out=gi[:], in0=gi[:],
                                        in1=off_c[:].rearrange("p a b -> p (a b)"),
                                        op=ALU.add)
                # merge + masked index extraction, top-8 then ranks 9-16
                v1 = sb.tile([128, 8], F32, tag="v1")
                v2 = sb.tile([128, 8], F32, tag="v2")
                cv2 = sb.tile([128, 64], F32, tag="cv2")
                cv3 = sb.tile([128, 64], F32, tag="cv3")
                mk = sb.tile([128, 64], F32, tag="mk")
                ex = sb.tile([128, 64], F32, tag="ex")
                i1f = sb.tile([128, 8], F32, tag="i1f")
                i2f = sb.tile([128, 8], F32, tag="i2f")
                nc.vector.max(out=v1[:], in_=cv[:])
                nc.vector.match_replace(out=cv2[:], in_to_replace=v1[:],
                                        in_values=cv[:], imm_value=-3e38)
                nc.vector.tensor_tensor(out=mk[:], in0=cv[:], in1=cv2[:],
                                        op=ALU.not_equal)
                nc.gpsimd.tensor_tensor(out=ex[:], in0=mk[:], in1=gi[:],
                                        op=ALU.mult)
                nc.vector.max(out=i1f[:], in_=ex[:])
                nc.scalar.activation(itile[:, 0:8], i1f[:], AF.Copy, bias=-1.0)
                nc.vector.max(out=v2[:], in_=cv2[:])
                nc.vector.match_replace(out=cv3[:], in_to_replace=v2[:],
                                        in_values=cv2[:], imm_value=-3e38)
                nc.vector.tensor_tensor(out=mk[:], in0=cv2[:], in1=cv3[:],
                                        op=ALU.not_equal)
                nc.gpsimd.tensor_tensor(out=ex[:], in0=mk[:], in1=gi[:],
                                        op=ALU.mult)
                nc.vector.max(out=i2f[:], in_=ex[:])
                nc.scalar.activation(itile[:, 8:16], i2f[:], AF.Copy, bias=-1.0)

            def consume(te, gt):
                m_t = sb.tile([128, D], F32, tag="m_t")
                if bdt == F16:
                    # k=16 slot-max as fp16 pairwise tree (2x DVE mode)
                    t1 = sb.tile([128, 8, D], F16, tag="t1")
                    nc.vector.tensor_tensor(out=t1[:], in0=gt[:, 0:8, :],
                                            in1=gt[:, 8:16, :], op=ALU.max)
                    t2 = sb.tile([128, 4, D], F16, tag="t2")
                    nc.vector.tensor_tensor(out=t2[:], in0=t1[:, 0:4, :],
                                            in1=t1[:, 4:8, :], op=ALU.max)
                    t3 = sb.tile([128, 2, D], F16, tag="t3")
                    nc.vector.tensor_tensor(out=t3[:], in0=t2[:, 0:2, :],
                                            in1=t2[:, 2:4, :], op=ALU.max)
                    nc.vector.tensor_tensor(out=m_t[:], in0=t3[:, 0, :],
                                            in1=t3[:, 1, :], op=ALU.max)
                else:
                    nc.vector.tensor_reduce(
                        out=m_t[:], in_=gt[:].rearrange("p k d -> p d k"),
                        op=mybir.AluOpType.max, axis=mybir.AxisListType.X)
                # epilogue: M^T transpose PSUM-accumulates with A^T
                # (= Wc^T @ x^T recomputed per tile), then BN+relu on ACT
                idt = ident
                for dc in range(0, D, 128):
                    dw = min(128, D - dc)
                    dst, dst_off = dsts[dc // 128]
                    mtp = ps.tile([128, 128], F32, tag="pscratch")
                    nc.tensor.matmul(mtp[0:dw, :], m_t[:, dc:dc + dw], idt[:],
                                     is_transpose=True, start=True, stop=False)
                    nc.tensor.matmul(mtp[0:dw, :], wc[:, dc:dc + dw],
                                     xT[:, te * 128:(te + 1) * 128],
                                     start=False, stop=True,
                                     skip_group_check=True)
                    nc.scalar.activation(
                        dst[dst_off:dst_off + dw, te * 128:(te + 1) * 128],
                        mtp[0:dw, :], AF.Relu,
                        scale=s_sb[0:dw, dc // 128:dc // 128 + 1],
                        bias=t_sb[0:dw, dc // 128:dc // 128 + 1])
                # next block's pd operands, one 512-col slice at a time
                if blk < 4 and te % 4 == 3:
                    if blk + 1 not in prepped:
                        prepped[blk + 1] = make_prep(blk + 1)
                    prepped[blk + 1][1](te // 4)
                # block4: chase each tile with its final concat matmul
                if blk == 4:
                    hp = ps.tile([128, 512], F32, tag="h5")
                    for kc, (src, kw) in enumerate(kchunks):
                        nc.tensor.matmul(
                            hp[:], src[0:kw, te * 128:(te + 1) * 128],
                            w5[0:kw, kc, :], start=(kc == 0), stop=False)
                    nc.tensor.matmul(hp[:], ones16[:], t5[:],
                                     start=False, stop=True,
                                     skip_group_check=True)
                    o_sb = sb.tile([128, 512], F32, tag="o_sb")
                    nc.scalar.activation(o_sb[:], hp[:], AF.Relu)
                    nc.sync.dma_start(out_d.ap()[te * 128:(te + 1) * 128, :],
                                      o_sb[:])

            pending = []
            for batch in BATCHES:
                wb = len(batch)
                itile_b = sb.tile([128, wb, 16], U16, tag="itile_b")
                iw_b = sb.tile([128, wb * 128], I16, tag="iw_b")
                for t in batch:
                    scan(t, itile_b[:, t - batch[0], :])
                # batched wrap over wb tiles:
                #   iw_b[q, tt*128 + m*8 + g] = itile_b[g*16+q, tt, m]
                # replicas all read only [0:16] (written by the SP-queue
                # wraps) — never a chained read of a just-replicated region,
                # which hard-crashes this runner.
                itb16 = itile_b[:].bitcast(I16)
                for g in range(8):
                    nc.sync.dma_start(
                        iw_b[0:16, g:g + 8 * (wb * 16 - 1) + 1:8]
                            .rearrange("p (tt m) -> p tt m", tt=wb),
                        itb16[g * 16:(g + 1) * 16, :, :])
                for r in range(1, 8):
                    nc.scalar.dma_start(iw_b[16 * r:16 * (r + 1), :],
                                        iw_b[0:16, :])
                gts = []
                for te in batch:
                    gt = sb3.tile([128, 16, D], bdt, tag="gt")
                    nc.gpsimd.dma_gather(
                        out_ap=gt[:], in_ap=bd_ap,
                        idxs_ap=iw_b[:, (te - batch[0]) * 128:(te - batch[0] + 1) * 128],
                        num_idxs=N, num_idxs_reg=N, elem_size=D,
                        single_packet=False)
                    gts.append((te, gt))
                # pre-emit the NEXT batch's first pd tiles ahead of the
                # pending consumes: their PSUM evictions otherwise queue
                # behind the consumes' BN/epilogue work in ACT program
                # order and stall the next batch's first scans (measured
                # 6-11us DVE gaps at every batch boundary)
                nb = BATCHES[BATCHES.index(batch) + 1] if batch is not BATCHES[-1] else []
                for t in nb[0:2]:
                    pre[t] = emit_pd(t)
                # consume the PREVIOUS batch while this batch's gathers fly
                for te, gt in pending:
                    consume(te, gt)
                pending = gts
            for te, gt in pending:
                consume(te, gt)

    return nc


_CACHED = {}


def _get_nc():
    if "nc" not in _CACHED:
        nc = bacc.Bacc("TRN2", target_bir_lowering=False, debug=False)
        build(nc)
        nc.compile()
        _CACHED["nc"] = nc
    return _CACHED["nc"]


def _in_maps(inputs):
    x = np.asarray(inputs["x"], dtype=np.float32)  # [8, 2048, 3]
    B = x.shape[0]
    common = {}
    for i, (C, D) in enumerate(BLOCKS):
        j = i + 1
        W = np.asarray(inputs[f"W{j}"], dtype=np.float32)
        g = np.asarray(inputs[f"g{j}"], dtype=np.float32)
        b = np.asarray(inputs[f"b{j}"], dtype=np.float32)
        m = np.asarray(inputs[f"m{j}"], dtype=np.float32)
        v = np.asarray(inputs[f"v{j}"], dtype=np.float32)
        s = (g / np.sqrt(v + EPS)).astype(np.float32)
        t = (b - m * s).astype(np.float32)
        assert (s > 0).all()
        common[f"Wc{j}"] = np.ascontiguousarray(W[:C])
        common[f"Wn{j}"] = np.ascontiguousarray(W[C:])
        common[f"s{j}"] = s.reshape(D, 1)
        common[f"t{j}"] = t.reshape(D, 1)
    W5 = np.asarray(inputs["W5"], dtype=np.float32)
    g5 = np.asarray(inputs["g5"], dtype=np.float32)
    b5 = np.asarray(inputs["b5"], dtype=np.float32)
    m5 = np.asarray(inputs["m5"], dtype=np.float32)
    v5 = np.asarray(inputs["v5"], dtype=np.float32)
    s5 = (g5 / np.sqrt(v5 + EPS)).astype(np.float32)
    t5 = (b5 - m5 * s5).astype(np.float32)
    W5s = (W5 * s5[None, :]).astype(np.float32)
    # K-chunks of the concat input: x1[0:64], x2[64:128], x3[128:256],
    # x4a[256:384], x4b[384:512]; pad the 64-row chunks to 128 partitions
    w5c = np.zeros((5, 128, 512), dtype=np.float32)
    w5c[0, 0:64] = W5s[0:64]
    w5c[1, 0:64] = W5s[64:128]
    w5c[2] = W5s[128:256]
    w5c[3] = W5s[256:384]
    w5c[4] = W5s[384:512]
    common["W5c"] = np.ascontiguousarray(
        w5c.transpose(1, 0, 2).reshape(128, 5 * 512)).astype(np.float16)
    common["t5"] = t5.reshape(1, 512).astype(np.float16)
    common["ones16"] = np.ones((1, 128), dtype=np.float16)
    maps = []
    for c in range(B):
        mp = dict(common)
        mp["xT"] = np.ascontiguousarray(x[c].T)
        maps.append(mp)
    return maps


def kernel(**inputs) -> np.ndarray:
    nc = _get_nc()
    maps = _in_maps(inputs)
    res = bass_utils.run_bass_kernel_spmd(nc, maps, core_ids=list(range(len(maps))))
    out = np.stack([r["out"] for r in res.results])  # [8, 2048, 512]
    return out.astype(np.float32)


if __name__ == "__main__":
    _get_nc()
    print("compiled ok")



# revision 27
# speedup vs baseline: 1.0424x; 1.0424x over previous
"""DGCNN Bass/Tile kernel for Trainium2 — 8-core data-parallel (1 point cloud per core).

Per edge-conv block (exact algebra):
  edge feat [ctr, nbr] @ W = ctr @ Wc + nbr @ Wn
  out[n] = max_k relu(bn(A[n] + B[idx[n,k]])) = relu((A[n] + max_k B[idx[n,k]]) * s + t)
  (s = g*rsqrt(v+eps) > 0, t = b - m*s; relu/max/affine commute since s > 0)

k-NN scores (monotone-equivalent to the reference's pd, per row):
  score[n, m] = 2<x_n, x_m> - |x_m|^2    (row-constant -|x_n|^2 dropped)
computed on PE (fp32) into PSUM, evicted by ACT.

Top-16 selection reads each 2048-wide row exactly TWICE on DVE:
  pass 1: 8 chunk-local max8 ops (256-wide) -> candidate values cv [128, 64]
  pass 2: 8 chunk-local max_index ops       -> chunk-local indices ci
Then everything runs on the 64-wide candidate arrays: gi = ci + 256*chunk + 1
(fp32); max8(cv) -> top-8 values; match_replace marks their cv positions;
(cv != replaced) * gi -> max8 extracts the global indices themselves; repeat
once on the replaced array for ranks 9-16. Exact unless one 256-chunk holds
>8 of a row's top-16 (verified < 3e-3 final rel err on the fixed dataset).

The pd scores rotate through FOUR quarter-width PSUM buffers: ACT evicts
quarter q while the PE fills later quarters / the next tile, and each 512-col
eviction unblocks its two DVE scan chunks immediately (a single pd buffer
serialized PE vs ACT every tile). The gi-add and mask-multiply steps of the
index extraction run on GPSIMD (only add/mult tensor_tensor are walrus-legal
on Pool). The final concat matmul runs entirely in fp16 (x4 holders and fp16
copies of x1/x2/x3 feed only this 1x1 conv, so 4.9e-4 rounding is harmless
and fp16 matmuls stream at 1 cyc/row vs 4 for fp32); block 4's k=16 slot-max
is an fp16 pairwise max tree in the DVE 2x mode.

Gather of B = x @ Wn rows from DRAM via gpsimd dma_gather (int16 idx wrapped
into 16 partitions, replicated to the 8 Q7 cores). Wrap batches of 6/5/5
tiles are software-pipelined one batch ahead: batch k's gathers run while
batch k+1's tiles are scanned, and the 5-tile TAIL batches keep the last
gathers' latency hidden under earlier consumes at each block boundary
(measured better than 8/4/2/1/1, 6/4/3/3, 4x4x4x4, and host-precomputed
block-1 prep). The next batch's first two pd tiles are pre-emitted ahead of
the pending consumes so their PSUM evictions don't queue behind the
consumes' BN work in ACT program order.
Block 4's B table is fp16 (its features feed no further neighbor selection -
only the final 1x1 conv); blocks 1-3 keep fp32 B + tensor_reduce, since
feature precision feeds the next kNN graph.

NOTE for future optimization: float32r looks like a free 4x on wide fp32
matmuls in the cost model, and fp32r matmul arithmetic is bit-identical to
fp32 given the same input bytes - BUT every producer instruction must be
tagged float32r, and tagged writes (ACT *and* DMA) physically round the
stored values to ~2.4e-4 (tf32-like). That noise flips ~12% of rows' marginal
kNN picks (exact fp32 small-K scores flip ~none) and compounds to 7.7e-2
final error. Selection-feeding data can never pass through f32r storage.

Epilogue: the slot-max transpose PSUM-accumulates with A^T = Wc^T @ x^T
recomputed per tile (no DVE add, no persistent A^T buffer); fused BN+relu on
ACT writes the next block's x^T holder directly. The next block's -|x|^2 row
and 2x^T operands are produced per 512-column slice as soon as the epilogues
covering that slice land, and block 4's epilogue is chased per-tile by the
final concat matmul, so block boundaries stay tight.
"""
import numpy as np
from contextlib import ExitStack

import concourse.bass as bass
import concourse.mybir as mybir
import concourse.tile as tile
from concourse import bacc
from concourse import bass_utils
from concourse.masks import make_identity

N = 2048
K = 16
EPS = 1e-3
NT = N // 128  # 16 row-tiles
# large tail batches: the last gathers' latency hides under the previous
# tiles' consumes instead of being fully exposed at each block boundary;
# the 6-buffer gather pool self-throttles the in-flight depth
BATCHES = [[0, 1, 2, 3, 4, 5], [6, 7, 8, 9, 10], [11, 12, 13, 14, 15]]
BLOCKS = [(3, 64), (64, 64), (64, 128), (128, 256)]  # (C_in, D_out)

F32 = mybir.dt.float32
F16 = mybir.dt.float16
U16 = mybir.dt.uint16
I16 = mybir.dt.int16
AF = mybir.ActivationFunctionType
ALU = mybir.AluOpType


def build(nc: bass.Bass):
    # ---- DRAM I/O ----
    xT_d = nc.dram_tensor("xT", [3, N], F32, kind="ExternalInput")
    wc_d, wn_d, s_d, t_d = [], [], [], []
    for i, (C, D) in enumerate(BLOCKS):
        wc_d.append(nc.dram_tensor(f"Wc{i+1}", [C, D], F32, kind="ExternalInput"))
        wn_d.append(nc.dram_tensor(f"Wn{i+1}", [C, D], F32, kind="ExternalInput"))
        s_d.append(nc.dram_tensor(f"s{i+1}", [D, 1], F32, kind="ExternalInput"))
        t_d.append(nc.dram_tensor(f"t{i+1}", [D, 1], F32, kind="ExternalInput"))
    w5_d = nc.dram_tensor("W5c", [128, 5 * 512], F16, kind="ExternalInput")
    t5_d = nc.dram_tensor("t5", [1, 512], F16, kind="ExternalInput")
    ones16_d = nc.dram_tensor("ones16", [1, 128], F16, kind="ExternalInput")
    out_d = nc.dram_tensor("out", [N, 512], F32, kind="ExternalOutput")
    b_d = [nc.dram_tensor(f"bdram{i+1}", [N, D], F16 if i == 3 else F32,
                          kind="Internal")
           for i, (C, D) in enumerate(BLOCKS)]

    with tile.TileContext(nc) as tc, ExitStack() as ctx:
        sb = ctx.enter_context(tc.tile_pool(name="sb", bufs=2))
        sb1 = ctx.enter_context(tc.tile_pool(name="sb1", bufs=1))
        ps = ctx.enter_context(tc.tile_pool(name="ps", bufs=2, space="PSUM"))
        psd = ctx.enter_context(tc.tile_pool(name="psd", bufs=4, space="PSUM"))
        sb3 = ctx.enter_context(tc.tile_pool(name="sb3", bufs=7))

        ident = sb1.tile([128, 128], F32, tag="ident")
        make_identity(nc, ident[:])
        ident16 = sb1.tile([128, 128], F16, tag="ident16")
        nc.scalar.activation(ident16[:], ident[:], AF.Copy)
        ones_row = sb1.tile([1, N], F32, tag="ones_row")
        nc.gpsimd.memset(ones_row[:], 1.0)
        ones_col = sb1.tile([128, 1], F32, tag="ones_col")
        nc.gpsimd.memset(ones_col[:], 1.0)
        # candidate -> global index offset: off[c*8+s] = 256*c + 1
        off_c = sb1.tile([128, 8, 8], F32, tag="off_c")
        nc.gpsimd.iota(off_c[:], [[256, 8], [0, 8]], base=1,
                       channel_multiplier=0,
                       allow_small_or_imprecise_dtypes=True)

        # persistent x^T holders for block4 output (final concat matmul only
        # -> fp16; the 4.9e-4 rounding feeds nothing downstream but the 1x1)
        x4aT = sb1.tile([128, N], F16, tag="x4aT")
        x4bT = sb1.tile([128, N], F16, tag="x4bT")

        # per-block Cc holders: rows 0:C = x^T, row C = -sq (block4: sq4)
        cc1 = sb1.tile([4, N], F32, tag="cc1")
        cc2 = sb1.tile([65, N], F32, tag="cc2")
        cc3 = sb1.tile([65, N], F32, tag="cc3")
        cc4 = sb1.tile([128, N], F32, tag="cc4")
        sq4 = sb1.tile([1, N], F32, tag="sq4")
        # fp16 copies of x1/x2/x3 for the final concat matmul (values only
        # feed the 1x1 conv; 4.9e-4 rounding is harmless, fp16 matmul is
        # 1 cyc/row vs 4 for fp32)
        cch2 = sb1.tile([64, N], F16, tag="cch2")
        cch3 = sb1.tile([64, N], F16, tag="cch3")
        cch4 = sb1.tile([128, N], F16, tag="cch4")

        nc.sync.dma_start(cc1[0:3, :], xT_d.ap())
        w5 = sb1.tile([128, 5, 512], F16, tag="w5")
        nc.sync.dma_start(w5[:], w5_d.ap().rearrange("p (a d) -> p a d", a=5))
        t5 = sb1.tile([1, 512], F16, tag="t5")
        nc.sync.dma_start(t5[:], t5_d.ap())
        ones16 = sb1.tile([1, 128], F16, tag="ones16")
        nc.sync.dma_start(ones16[:], ones16_d.ap())

        xT_of = {1: cc1, 2: cc2, 3: cc3, 4: cc4}
        cch_of = {2: cch2, 3: cch3, 4: cch4}
        kchunks = [(cch2, 64), (cch3, 64), (cch4, 128), (x4aT, 128), (x4bT, 128)]

        def make_prep(blk):
            """Per-512-col-slice prep of block blk's pd operands: the -|x|^2
            row (DMA'd into the aug row) and rr = 2x^T."""
            C, D = BLOCKS[blk - 1]
            cct = xT_of[blk]
            xT = cct[0:C, :]
            rr = sb1.tile([C + 1, N] if blk < 4 else [128, N], F32,
                          tag=f"rr{blk % 2}")
            if blk < 4:
                nc.sync.dma_start(rr[C:C + 1, :], ones_row[:])

            def emit_slice(j):
                sl = slice(j * 512, (j + 1) * 512)
                xsq = sb.tile([C, 512], F32, tag="xsq")
                nc.scalar.activation(xsq[:], xT[:, sl], AF.Square)
                sqp = ps.tile([1, 512], F32, tag="pscratch")
                nc.tensor.matmul(sqp[:], ones_col[0:C, :], xsq[:],
                                 start=True, stop=True)
                sqstage = sb.tile([1, 512], F32, tag="sqstage")
                nc.scalar.activation(sqstage[:], sqp[:], AF.Copy, scale=-1.0)
                sq_dst = sq4[0:1, sl] if blk == 4 else cct[C:C + 1, sl]
                nc.sync.dma_start(sq_dst, sqstage[:])
                nc.scalar.mul(rr[0:C, sl], xT[:, sl], 2.0)
                if blk > 1:
                    nc.scalar.activation(cch_of[blk][0:C, sl], xT[:, sl], AF.Copy)

            return rr, emit_slice

        prepped = {1: make_prep(1)}
        for j in range(4):
            prepped[1][1](j)

        for i, (C, D) in enumerate(BLOCKS):
            blk = i + 1
            cct = xT_of[blk]
            xT = cct[0:C, :]
            bd_ap = b_d[i].ap()
            bdt = F16 if blk == 4 else F32
            rr = prepped[blk][0]

            if blk < 4:
                rr_chunks = [rr[0:C + 1, :]]
                cc_chunks = [cct[0:C + 1, :]]
            else:
                rr_chunks = [rr[:], ones_row[:]]
                cc_chunks = [cct[0:128, :], sq4[:]]
            nk = len(rr_chunks)

            def emit_pd(t):
                # four quarter-width PSUM buffers: ACT evicts quarter q while
                # the PE already fills the next quarters (and the next tile) —
                # a single buffer would serialize PE vs ACT every tile, and
                # each 512-col eviction unblocks its two scan chunks early
                pdsb = sb.tile([128, N], F32, tag="pdsb")
                for q in range(4):
                    pd = psd.tile([128, 512], F32, tag="pd")
                    for kc, (rc, cc) in enumerate(zip(rr_chunks, cc_chunks)):
                        nc.tensor.matmul(
                            pd[:],
                            rc[:, t * 128:(t + 1) * 128],
                            cc[:, q * 512:(q + 1) * 512],
                            start=(kc == 0), stop=(kc == nk - 1))
                    nc.scalar.activation(pdsb[:, q * 512:(q + 1) * 512],
                                         pd[:], AF.Copy)
                return pdsb

            # hoist pd of the first two tiles ahead of the weight/B prep so
            # the DVE scan starts while PE/ACT still run the block prep
            pre = {t: emit_pd(t) for t in (0, 1)}

            # ---- prep: weights / bn params ----
            wc = sb.tile([C, D], F32, tag="wc")
            wn = sb.tile([C, D], F32, tag="wn")
            nc.sync.dma_start(wc[:], wc_d[i].ap())
            nc.sync.dma_start(wn[:], wn_d[i].ap())
            nch = (D + 127) // 128
            s_sb = sb.tile([128, nch], F32, tag="s_sb")
            t_sb = sb.tile([128, nch], F32, tag="t_sb")
            for c in range(nch):
                dw_ = min(128, D - c * 128)
                nc.sync.dma_start(s_sb[0:dw_, c:c + 1], s_d[i].ap()[c * 128:c * 128 + dw_, :])
                nc.sync.dma_start(t_sb[0:dw_, c:c + 1], t_d[i].ap()[c * 128:c * 128 + dw_, :])

            # ---- prep: B = x @ Wn row-major -> DRAM (block4: fp16) ----
            for t in range(NT):
                bp = ps.tile([128, D], F32, tag="pscratch")
                nc.tensor.matmul(bp[:], xT[:, t * 128:(t + 1) * 128], wn[:],
                                 start=True, stop=True)
                b_sb = sb.tile([128, D], bdt, tag="b_sb")
                nc.scalar.activation(b_sb[:], bp[:], AF.Copy)
                nc.sync.dma_start(bd_ap[t * 128:(t + 1) * 128, :], b_sb[:])

            # ---- main loop ----
            if blk == 1:
                dsts = [(cc2, 0)]
            elif blk == 2:
                dsts = [(cc3, 0)]
            elif blk == 3:
                dsts = [(cc4, 0)]
            else:
                dsts = [(x4aT, 0), (x4bT, 0)]

            def scan(t, itile):
                pdsb = pre.pop(t, None)
                if pdsb is None:
                    pdsb = emit_pd(t)
                # pass 1+2: per-chunk top-8 values + chunk-local indices
                cv = sb.tile([128, 64], F32, tag="cv")
                ci = sb.tile([128, 64], U16, tag="ci")
                for c in range(8):
                    nc.vector.max(out=cv[:, c * 8:(c + 1) * 8],
                                  in_=pdsb[:, c * 256:(c + 1) * 256])
                    nc.vector.max_index(out=ci[:, c * 8:(c + 1) * 8],
                                        in_max=cv[:, c * 8:(c + 1) * 8],
                                        in_values=pdsb[:, c * 256:(c + 1) * 256])
                # gi = global index + 1, as exact fp32
                gi = sb.tile([128, 64], F32, tag="gi")
                nc.scalar.activation(gi[:], ci[:], AF.Copy)
                nc.gpsimd.tensor_tensor(out=gi[:], in0=gi[:],
                                        in1=off_c[:].rearrange("p a b -> p (a b)"),
                                        op=ALU.add)
                # merge + masked index extraction, top-8 then ranks 9-16
                v1 = sb.tile([128, 8], F32, tag="v1")
                v2 = sb.tile([128, 8], F32, tag="v2")
                cv2 = sb.tile([128, 64], F32, tag="cv2")
                cv3 = sb.tile([128, 64], F32, tag="cv3")
                mk = sb.tile([128, 64], F32, tag="mk")
                ex = sb.tile([128, 64], F32, tag="ex")
                i1f = sb.tile([128, 8], F32, tag="i1f")
                i2f = sb.tile([128, 8], F32, tag="i2f")
                nc.vector.max(out=v1[:], in_=cv[:])
                nc.vector.match_replace(out=cv2[:], in_to_replace=v1[:],
                                        in_values=cv[:], imm_value=-3e38)
                nc.vector.tensor_tensor(out=mk[:], in0=cv[:], in1=cv2[:],
                                        op=ALU.not_equal)
                nc.gpsimd.tensor_tensor(out=ex[:], in0=mk[:], in1=gi[:],
                                        op=ALU.mult)
                nc.vector.max(out=i1f[:], in_=ex[:])
                nc.scalar.activation(itile[:, 0:8], i1f[:], AF.Copy, bias=-1.0)
                nc.vector.max(out=v2[:], in_=cv2[:])
                nc.vector.match_replace(out=cv3[:], in_to_replace=v2[:],
                                        in_values=cv2[:], imm_value=-3e38)
                nc.vector.tensor_tensor(out=mk[:], in0=cv2[:], in1=cv3[:],
                                        op=ALU.not_equal)
                nc.gpsimd.tensor_tensor(out=ex[:], in0=mk[:], in1=gi[:],
                                        op=ALU.mult)
                nc.vector.max(out=i2f[:], in_=ex[:])
                nc.scalar.activation(itile[:, 8:16], i2f[:], AF.Copy, bias=-1.0)

            def consume(te, gt):
                m_t = sb.tile([128, D], F32, tag="m_t")
                if bdt == F16:
                    # k=16 slot-max as fp16 pairwise tree (2x DVE mode)
                    t1 = sb.tile([128, 8, D], F16, tag="t1")
                    nc.vector.tensor_tensor(out=t1[:], in0=gt[:, 0:8, :],
                                            in1=gt[:, 8:16, :], op=ALU.max)
                    t2 = sb.tile([128, 4, D], F16, tag="t2")
                    nc.vector.tensor_tensor(out=t2[:], in0=t1[:, 0:4, :],
                                            in1=t1[:, 4:8, :], op=ALU.max)
                    t3 = sb.tile([128, 2, D], F16, tag="t3")
                    nc.vector.tensor_tensor(out=t3[:], in0=t2[:, 0:2, :],
                                            in1=t2[:, 2:4, :], op=ALU.max)
                    nc.vector.tensor_tensor(out=m_t[:], in0=t3[:, 0, :],
                                            in1=t3[:, 1, :], op=ALU.max)
                else:
                    nc.vector.tensor_reduce(
                        out=m_t[:], in_=gt[:].rearrange("p k d -> p d k"),
                        op=mybir.AluOpType.max, axis=mybir.AxisListType.X)
                # epilogue: M^T transpose PSUM-accumulates with A^T
                # (= Wc^T @ x^T recomputed per tile), then BN+relu on ACT
                idt = ident
                for dc in range(0, D, 128):
                    dw = min(128, D - dc)
                    dst, dst_off = dsts[dc // 128]
                    mtp = ps.tile([128, 128], F32, tag="pscratch")
                    nc.tensor.matmul(mtp[0:dw, :], m_t[:, dc:dc + dw], idt[:],
                                     is_transpose=True, start=True, stop=False)
                    nc.tensor.matmul(mtp[0:dw, :], wc[:, dc:dc + dw],
                                     xT[:, te * 128:(te + 1) * 128],
                                     start=False, stop=True,
                                     skip_group_check=True)
                    nc.scalar.activation(
                        dst[dst_off:dst_off + dw, te * 128:(te + 1) * 128],
                        mtp[0:dw, :], AF.Relu,
                        scale=s_sb[0:dw, dc // 128:dc // 128 + 1],
                        bias=t_sb[0:dw, dc // 128:dc // 128 + 1])
                # next block's pd operands, one 512-col slice at a time
                if blk < 4 and te % 4 == 3:
                    if blk + 1 not in prepped:
                        prepped[blk + 1] = make_prep(blk + 1)
                    prepped[blk + 1][1](te // 4)
                # block4: chase each tile with its final concat matmul
                if blk == 4:
                    hp = ps.tile([128, 512], F32, tag="h5")
                    for kc, (src, kw) in enumerate(kchunks):
                        nc.tensor.matmul(
                            hp[:], src[0:kw, te * 128:(te + 1) * 128],
                            w5[0:kw, kc, :], start=(kc == 0), stop=False)
                    nc.tensor.matmul(hp[:], ones16[:], t5[:],
                                     start=False, stop=True,
                                     skip_group_check=True)
                    o_sb = sb.tile([128, 512], F32, tag="o_sb")
                    nc.scalar.activation(o_sb[:], hp[:], AF.Relu)
                    nc.sync.dma_start(out_d.ap()[te * 128:(te + 1) * 128, :],
                                      o_sb[:])

            pending = []
            for batch in BATCHES:
                wb = len(batch)
                itile_b = sb.tile([128, wb, 16], U16, tag="itile_b")
                iw_b = sb.tile([128, wb * 128], I16, tag="iw_b")
                for t in batch:
                    scan(t, itile_b[:, t - batch[0], :])
                # batched wrap over wb tiles:
                #   iw_b[q, tt*128 + m*8 + g] = itile_b[g*16+q, tt, m]
                # replicas all read only [0:16] (written by the SP-queue
                # wraps) — never a chained read of a just-replicated region,
                # which hard-crashes this runner.
                itb16 = itile_b[:].bitcast(I16)
                for g in range(8):
                    nc.sync.dma_start(
                        iw_b[0:16, g:g + 8 * (wb * 16 - 1) + 1:8]
                            .rearrange("p (tt m) -> p tt m", tt=wb),
                        itb16[g * 16:(g + 1) * 16, :, :])
                for r in range(1, 8):
                    nc.scalar.dma_start(iw_b[16 * r:16 * (r + 1), :],
                                        iw_b[0:16, :])
                gts = []
                for te in batch:
                    gt = sb3.tile([128, 16, D], bdt, tag="gt")
                    nc.gpsimd.dma_gather(
                        out_ap=gt[:], in_ap=bd_ap,
                        idxs_ap=iw_b[:, (te - batch[0]) * 128:(te - batch[0] + 1) * 128],
                        num_idxs=N, num_idxs_reg=N, elem_size=D,
                        single_packet=False)
                    gts.append((te, gt))
                # pre-emit the NEXT batch's first pd tiles ahead of the
                # pending consumes: their PSUM evictions otherwise queue
                # behind the consumes' BN/epilogue work in ACT program
                # order and stall the next batch's first scans (measured
                # 6-11us DVE gaps at every batch boundary)
                nb = BATCHES[BATCHES.index(batch) + 1] if batch is not BATCHES[-1] else []
                for t in nb[0:2]:
                    pre[t] = emit_pd(t)
                # consume the PREVIOUS batch while this batch's gathers fly
                for te, gt in pending:
                    consume(te, gt)
                pending = gts
            for te, gt in pending:
                consume(te, gt)

    return nc


_CACHED = {}


def _get_nc():
    if "nc" not in _CACHED:
        nc = bacc.Bacc("TRN2", target_bir_lowering=False, debug=False)
        build(nc)
        nc.compile()
        _CACHED["nc"] = nc
    return _CACHED["nc"]


def _in_maps(inputs):
    x = np.asarray(inputs["x"], dtype=np.float32)  # [8, 2048, 3]
    B = x.shape[0]
    common = {}
    for i, (C, D) in enumerate(BLOCKS):
        j = i + 1
        W = np.asarray(inputs[f"W{j}"], dtype=np.float32)
        g = np.asarray(inputs[f"g{j}"], dtype=np.float32)
        b = np.asarray(inputs[f"b{j}"], dtype=np.float32)
        m = np.asarray(inputs[f"m{j}"], dtype=np.float32)
        v = np.asarray(inputs[f"v{j}"], dtype=np.float32)
        s = (g / np.sqrt(v + EPS)).astype(np.float32)
        t = (b - m * s).astype(np.float32)
        assert (s > 0).all()
        common[f"Wc{j}"] = np.ascontiguousarray(W[:C])
        common[f"Wn{j}"] = np.ascontiguousarray(W[C:])
        common[f"s{j}"] = s.reshape(D, 1)
        common[f"t{j}"] = t.reshape(D, 1)
    W5 = np.asarray(inputs["W5"], dtype=np.float32)
    g5 = np.asarray(inputs["g5"], dtype=np.float32)
    b5 = np.asarray(inputs["b5"], dtype=np.float32)
    m5 = np.asarray(inputs["m5"], dtype=np.float32)
    v5 = np.asarray(inputs["v5"], dtype=np.float32)
    s5 = (g5 / np.sqrt(v5 + EPS)).astype(np.float32)
    t5 = (b5 - m5 * s5).astype(np.float32)
    W5s = (W5 * s5[None, :]).astype(np.float32)
    # K-chunks of the concat input: x1[0:64], x2[64:128], x3[128:256],
    # x4a[256:384], x4b[384:512]; pad the 64-row chunks to 128 partitions
    w5c = np.zeros((5, 128, 512), dtype=np.float32)
    w5c[0, 0:64] = W5s[0:64]
    w5c[1, 0:64] = W5s[64:128]
    w5c[2] = W5s[128:256]
    w5c[3] = W5s[256:384]
    w5c[4] = W5s[384:512]
    common["W5c"] = np.ascontiguousarray(
        w5c.transpose(1, 0, 2).reshape(128, 5 * 512)).astype(np.float16)
    common["t5"] = t5.reshape(1, 512).astype(np.float16)
    common["ones16"] = np.ones((1, 128), dtype=np.float16)
    maps = []
    for c in range(B):
        mp = dict(common)
        mp["xT"] = np.ascontiguousarray(x[c].T)
        maps.append(mp)
    return maps


def kernel(**inputs) -> np.ndarray:
    nc = _get_nc()
    maps = _in_maps(inputs)
    res = bass_utils.run_bass_kernel_spmd(nc, maps, core_ids=list(range(len(maps))))
    out = np.stack([r["out"] for r in res.results])  # [8, 2048, 512]
    return out.astype(np.float32)


if __name__ == "__main__":
    _get_nc()
    print("compiled ok")

